# revision 32
# baseline (speedup 1.0000x reference)
"""EdgeConv2dDiff Trainium2 Bass kernel.

Reference computation (B=1, C=64, N=50000, K=16, COUT=64):
    e = concat([x_i, x_j - x_i], axis=channel)          # [B, 2C, N, K]
    y = relu(einsum("bcnk,oc->bonk", e, W) + b)          # [B, COUT, N, K]
    out = max(y, axis=K, keepdims=True)                  # [B, COUT, N, 1]

Algebraic restructuring used here:
    W1 @ x_i + W2 @ (x_j - x_i) == (W1 - W2) @ x_i + W2 @ x_j
so the folded weight  wT = [[(W1-W2).T], [W2.T]]  ([2C, COUT]) turns the
whole edge-feature construction into a single 128-contraction matmul over
a stacked input [x_i; x_j] ([2C, N*K]).  Also
    max_k(relu(z_k + b)) == relu(max_k(z_k) + b)
so the K-max runs on raw PSUM output and bias+relu touches 16x fewer
elements.

Sharding: data-parallel over nodes N across 8 cores (6250 nodes each),
no cross-core communication.

Per-core structure: the core's 6250 nodes are split into two halves of
3125; two input DMA streams (one per half) run in lockstep.  Each PSUM
tile takes a 32-node matmul from stream A on partitions 0:63 and the
matching 32-node matmul from stream B on partitions 64:127, so every
vector K-max reduce covers all 128 partitions.  Results accumulate into
a persistent SBUF tile ([128, 3125]: row p<64 = channel p of half A,
row 64+p = channel p of half B) that is flushed with a few large
contiguous-run output DMAs.
"""

import sys

import numpy as np

for _p in ("/opt/trn_rl_repo",):
    if _p not in sys.path:
        sys.path.insert(0, _p)

B, C, N, K = 1, 64, 50000, 16
COUT = 64
NCORES = 8
NS = N // NCORES          # 6250 nodes per core
NSH = NS // 2             # 3125 nodes per half-stream
FS = NS * K               # 100000 matmul columns per core
FSH = NSH * K             # 50000 columns per half-stream
CHUNK_NODES = 256         # nodes per DMA chunk per stream ([128,4096]=2MB)
TAIL_NODES = 128          # chunk size for the drain-sensitive tail
MM_NODES = 32             # nodes per matmul (32*16 = 512 = max fp32 free)

_CACHE = {}


def _chunk_schedule():
    """Per-half-stream chunk sizes: big chunks first, small at the end so
    the post-last-DMA compute drain is short."""
    chunks = []
    rem = NSH
    while rem > CHUNK_NODES + 4 * TAIL_NODES:
        chunks.append(CHUNK_NODES)
        rem -= CHUNK_NODES
    while rem > 0:
        c = min(TAIL_NODES, rem)
        chunks.append(c)
        rem -= c
    return chunks


def _build():
    if "nc" in _CACHE:
        return _CACHE["nc"]
    import concourse.bacc as bacc
    import concourse.mybir as mybir
    from concourse.tile import TileContext

    fp32 = mybir.dt.float32
    nc = bacc.Bacc(
        "TRN2", target_bir_lowering=False, debug=False, num_devices=NCORES
    )
    x = nc.dram_tensor("x", [2 * C, FS], fp32, kind="ExternalInput")
    wT = nc.dram_tensor("wT", [2 * C, COUT], fp32, kind="ExternalInput")
    bias = nc.dram_tensor("bias", [2 * C, 1], fp32, kind="ExternalInput")
    y = nc.dram_tensor("y", [COUT, NS], fp32, kind="ExternalOutput")

    chunks = _chunk_schedule()
    # flush output mid-stream so the final post-compute flush is tiny
    n_chunks = len(chunks)
    flush_points = {n_chunks // 2 - 1, n_chunks - 4, n_chunks - 2, n_chunks - 1}

    with TileContext(nc) as tc:
        with (
            tc.tile_pool(name="const", bufs=1) as cpool,
            tc.tile_pool(name="xin", bufs=3) as xpool,
            tc.tile_pool(name="psum", bufs=8, space="PSUM") as ppool,
            tc.tile_pool(name="oacc", bufs=1) as opool,
        ):
            wt = cpool.tile([2 * C, COUT], fp32)
            bt = cpool.tile([2 * C, 1], fp32)
            oacc = opool.tile([2 * C, NSH], fp32)
            # both half-streams as one 3D view: [p, half, col]
            xv = x.rearrange("p (h q) -> p h q", h=2)

            first = True
            node = 0  # offset within the half-stream
            flushed = 0
            for ci, nn_ in enumerate(chunks):
                cols = nn_ * K
                # one fused DMA per chunk: half A lands at tile cols
                # [0, cols), half B at [cols, 2*cols)
                xt = xpool.tile([2 * C, 2 * CHUNK_NODES * K], fp32, tag="xt")
                nc.sync.dma_start(
                    xt[:, : 2 * cols],
                    xv[:, :, node * K : node * K + cols],
                )
                if first:
                    # constants after the first big DMAs so the input
                    # stream starts as early as possible
                    nc.sync.dma_start(wt[:], wT[:])
                    nc.sync.dma_start(bt[:], bias[:])
                    first = False
                ngroups = (nn_ + MM_NODES - 1) // MM_NODES
                for t in range(ngroups):
                    g0 = t * MM_NODES
                    gn = min(MM_NODES, nn_ - g0)
                    ps = ppool.tile([2 * C, MM_NODES * K], fp32, tag="ps")
                    nc.tensor.matmul(
                        ps[0:COUT, : gn * K],
                        wt[:],
                        xt[:, g0 * K : (g0 + gn) * K],
                        start=True,
                        stop=True,
                    )
                    nc.tensor.matmul(
                        ps[COUT : 2 * COUT, : gn * K],
                        wt[:],
                        xt[:, cols + g0 * K : cols + (g0 + gn) * K],
                        start=True,
                        stop=True,
                    )
                    nc.vector.tensor_reduce(
                        oacc[:, node + g0 : node + g0 + gn],
                        ps[:, : gn * K].rearrange("p (n k) -> p n k", k=K),
                        axis=mybir.AxisListType.X,
                        op=mybir.AluOpType.max,
                    )
                nc.scalar.activation(
                    oacc[:, node : node + nn_],
                    oacc[:, node : node + nn_],
                    mybir.ActivationFunctionType.Relu,
                    bias=bt[:],
                    scale=1.0,
                )
                node += nn_
                if ci in flush_points:
                    nc.sync.dma_start(
                        y[:, flushed:node], oacc[0:COUT, flushed:node]
                    )
                    nc.sync.dma_start(
                        y[:, NSH + flushed : NSH + node],
                        oacc[COUT : 2 * COUT, flushed:node],
                    )
                    flushed = node

    nc.compile()
    _CACHE["nc"] = nc
    return nc


def _prep_inputs(x_i, x_j, W, b):
    x_i = np.asarray(x_i, dtype=np.float32).reshape(C, N * K)
    x_j = np.asarray(x_j, dtype=np.float32).reshape(C, N * K)
    W = np.asarray(W, dtype=np.float32)
    b = np.asarray(b, dtype=np.float32)

    W1, W2 = W[:, :C], W[:, C:]
    wT = np.ascontiguousarray(
        np.concatenate([(W1 - W2).T, W2.T], axis=0)
    )  # [2C, COUT]
    bias = np.ascontiguousarray(
        np.concatenate([b, b]).reshape(2 * C, 1)
    )  # replicated onto both partition halves

    xfull = np.empty((NCORES, 2 * C, FS), dtype=np.float32)
    for s in range(NCORES):
        xfull[s, :C] = x_i[:, s * FS : (s + 1) * FS]
        xfull[s, C:] = x_j[:, s * FS : (s + 1) * FS]

    return [
        {"x": xfull[s], "wT": wT, "bias": bias} for s in range(NCORES)
    ]


def run(x_i, x_j, W, b, **spmd_kwargs):
    """Build + run, returning (full_output, BassKernelResults)."""
    from concourse.bass_utils import run_bass_kernel_spmd

    nc = _build()
    in_maps = _prep_inputs(x_i, x_j, W, b)
    res = run_bass_kernel_spmd(nc, in_maps, list(range(NCORES)), **spmd_kwargs)
    y = np.concatenate(
        [res.results[s]["y"] for s in range(NCORES)], axis=1
    )  # [COUT, N]
    return y.reshape(B, COUT, N, 1), res


def kernel(x_i, x_j, W, b):
    out, _ = run(x_i, x_j, W, b)
    return out
